# revision 1
# baseline (speedup 1.0000x reference)
"""DARTS-cell (moe_routing) Trainium2 kernel.

Strategy: data-parallel over batch B=32 across 8 cores (4 samples/core).
Per-sample top-2-of-8 gating (alphas) computed on host; zero-alpha branches
contribute exactly zero (dense mode) or are skipped (sparse mode).

Layout on device: channel-major [C=128 partitions, H*W=1024 free] per sample.
- 1x1 convs + preprocess: PE matmuls in float32r (full-rate fp32, ~12-bit
  mantissa rounding) accumulating into per-(step,sample) PSUM state.
- depthwise convs: per-partition-scalar shifted MACs (scalar_tensor_tensor)
  on DVE/GpSimd over zero-padded relu'd buffers built by ScalarE
  (activation Relu with per-sample alpha folded into the scale).
- pools: shifted tensor_max/tensor_add trees; avg uses a precomputed
  BN/count map; pool results + skip are added into state PSUM by DVE after
  all matmuls (PSUM has_written rule).
- BN (eval, affine=False) folded into weights/maps on host.
"""

import sys

sys.path.insert(0, "/opt/trn_rl_repo")

import numpy as np
from concourse import bacc, mybir, tile
from concourse.bass_utils import run_bass_kernel_spmd

STEPS = 4
N_MIX = 14
OFFSETS = [0, 2, 5, 9]
B, C_IN, C, H, W = 32, 512, 128, 32, 32
HW = H * W
N_CORES = 8
BL = B // N_CORES  # samples per core
BN_SCALE = float(1.0 / np.sqrt(1.0 + 1e-5))

F32 = mybir.dt.float32
F32R = mybir.dt.float32r
ALU = mybir.AluOpType
ACTF = mybir.ActivationFunctionType

# branch op indices in PRIMITIVES order
O_MAX, O_AVG, O_SKIP, O_SEP3, O_SEP5, O_DIL3, O_DIL5 = 1, 2, 3, 4, 5, 6, 7

# dw tap table layout per mixed-op m: [sep3_dw1(9), sep3_dw2(9), sep5_dw1(25),
# sep5_dw2(25), dil3(9), dil5(25)] -> 102 taps
TAP_OFF = {"s3a": 0, "s3b": 9, "d3": 18}
N_TAPS = 27
# pw matrix slots per m (DVE branches only); sep5/dil5 are PE-fused
PW_SLOT = {"s3a": 0, "s3b": 1, "d3": 2}
N_PW = 3


def _host_alphas(gates, top):
    """Per-sample masked-softmax over top-k gate entries. gates [N_MIX,B,8]."""
    g = gates.astype(np.float64)
    idx = np.argsort(-g, axis=-1, kind="stable")[..., :top]  # [m,b,top]
    mask = np.zeros(g.shape, bool)
    np.put_along_axis(mask, idx, True, axis=-1)
    gm = np.where(mask, g, -np.inf)
    gm -= gm.max(axis=-1, keepdims=True)
    e = np.exp(gm)
    p = e / e.sum(axis=-1, keepdims=True)
    return p.astype(np.float32)  # exact zeros off top-k


def build_program(active, n_cores=N_CORES):
    """active[(m, b_local)] -> iterable of branch op indices (1..7) to emit.
    Must be the same for every core (SPMD); dense mode passes all 7."""
    nc = bacc.Bacc("TRN2", target_bir_lowering=False, debug=False,
                   num_devices=n_cores)

    x0_d = nc.dram_tensor("x0", [BL, 4, 128, HW], F32, kind="ExternalInput").ap()
    x1_d = nc.dram_tensor("x1", [BL, 4, 128, HW], F32, kind="ExternalInput").ap()
    prew_d = nc.dram_tensor("prew", [128, 2, 4, 128], F32R, kind="ExternalInput").ap()
    pw_d = nc.dram_tensor("pw", [128, N_MIX, N_PW, 128], F32R, kind="ExternalInput").ap()
    fw5a_d = nc.dram_tensor("fw5a", [128, N_MIX, 25, 128], F32R, kind="ExternalInput").ap()
    fw5b_d = nc.dram_tensor("fw5b", [128, N_MIX, 25, 128], F32R, kind="ExternalInput").ap()
    fwd5_d = nc.dram_tensor("fwd5", [128, N_MIX, 25, 128], F32R, kind="ExternalInput").ap()
    dwt_d = nc.dram_tensor("dwt", [128, N_MIX, N_TAPS], F32, kind="ExternalInput").ap()
    alf_d = nc.dram_tensor("alf", [128, N_MIX, BL, 8], F32, kind="ExternalInput").ap()
    rmap_d = nc.dram_tensor("rmap", [128, 32, 32], F32, kind="ExternalInput").ap()
    out_d = nc.dram_tensor("out", [BL, 4, 128, HW], F32R, kind="ExternalOutput").ap()

    with tile.TileContext(nc) as tc:
        with (
            tc.tile_pool(name="const", bufs=1) as cpool,
            tc.tile_pool(name="work", bufs=1) as wpool,
            tc.tile_pool(name="xs", bufs=2) as xpool,
            tc.tile_pool(name="dwa", bufs=2) as dpool,
            tc.tile_pool(name="ps_state", bufs=2, space="PSUM") as pspool,
            tc.tile_pool(name="ps_scr", bufs=2, space="PSUM") as scrpool,
            tc.tile_pool(name="fw", bufs=2) as fwpool,
        ):
            # ---- constants / weights ----
            prew = cpool.tile([128, 2, 4, 128], F32R, tag="prew")
            pw = cpool.tile([128, N_MIX, N_PW, 128], F32R, tag="pw")
            dwt = cpool.tile([128, N_MIX, N_TAPS], F32, tag="dwt")
            alf = cpool.tile([128, N_MIX, BL, 8], F32, tag="alf")
            rmap = cpool.tile([128, 32, 32], F32, tag="rmap")
            nc.sync.dma_start(prew[:], prew_d)
            nc.sync.dma_start(pw[:], pw_d)
            nc.sync.dma_start(dwt[:], dwt_d)
            nc.sync.dma_start(alf[:], alf_d)
            nc.sync.dma_start(rmap[:], rmap_d)

            # ---- persistent padded work buffers ----
            z34 = [wpool.tile([128, 34, 34], F32R, tag=f"z34_{i}", name=f"z34_{i}") for i in range(2)]
            z36 = [wpool.tile([128, 36, 36], F32R, tag=f"z36_{i}", name=f"z36_{i}") for i in range(2)]
            z40 = [wpool.tile([128, 40, 40], F32R, tag=f"z40_{i}", name=f"z40_{i}") for i in range(2)]
            xpmax = wpool.tile([128, 34, 34], F32, tag="xpmax")
            xpsum = wpool.tile([128, 34, 34], F32, tag="xpsum")
            rmpad = wpool.tile([128, 34, 32], F32, tag="rmpad")
            rspad = wpool.tile([128, 34, 32], F32, tag="rspad")
            ptmp = [wpool.tile([128, 32, 32], F32, tag=f"ptmp_{i}", name=f"ptmp_{i}") for i in range(2)]

            states = wpool.tile([128, 6, 32, 32], F32R, tag="states")
            pooled = wpool.tile([128, 2, 5, 32, 32], F32, tag="pooled")

            for z in z34 + z36 + z40:
                nc.gpsimd.memset(z[:].bitcast(F32), 0.0)
            nc.gpsimd.memset(xpmax[:], -1e30)
            nc.gpsimd.memset(xpsum[:], 0.0)
            nc.gpsimd.memset(rmpad[:], -1e30)
            nc.gpsimd.memset(rspad[:], 0.0)

            zpad_for = {  # branch -> (buffers, pad, tap stride)
                O_SEP3: (z34, 1, 1),
                O_SEP5: (z36, 2, 1),
                O_DIL3: (z40, 2, 2),
                O_DIL5: (z40, 4, 2),
            }

            def flat(ap3):  # [128, a, b] -> [128, a*b]
                return ap3.rearrange("p a b -> p (a b)")

            def mm_chunks(psum3, lhsT, rhs3, flags):
                """two N=512 matmuls; flags = (start0, stop0, start1, stop1)."""
                s0, e0, s1, e1 = flags
                nc.tensor.matmul(psum3[:, 0:16, :], lhsT, rhs3[:, 0:16, :],
                                 start=s0, stop=e0)
                nc.tensor.matmul(psum3[:, 16:32, :], lhsT, rhs3[:, 16:32, :],
                                 start=s1, stop=e1)

            def dw_chain(eng, zt, dwacc, m, tap0, k, pad, stride, interior):
                """depthwise conv: dwacc = sum_t dwt[:,m,tap0+t] * shift_t(zt)."""
                first = True
                for ky in range(k):
                    for kx in range(k):
                        t = tap0 + ky * k + kx
                        y0 = interior - pad + stride * ky
                        x0 = interior - pad + stride * kx
                        view = zt[:, y0:y0 + 32, x0:x0 + 32]
                        sc = dwt[:, m, t:t + 1]
                        if first:
                            eng.tensor_scalar_mul(dwacc[:], view, sc)
                            first = False
                        else:
                            eng.scalar_tensor_tensor(
                                dwacc[:], view, sc, dwacc[:],
                                op0=ALU.mult, op1=ALU.add)

            def fused_stage(fw_d, m, zt, pad, stride, interior, k, psum3,
                            gfirst, glast):
                """depthwise+pointwise fused: accumulate k*k tap matmuls."""
                taps = k * k
                half = (taps + 1) // 2
                for (a, e) in ((0, half), (half, taps)):
                    fwt = fwpool.tile([128, 13, 128], F32R, tag="fw")
                    nc.sync.dma_start(fwt[:, 0:e - a, :], fw_d[:, m, a:e, :])
                    for t in range(a, e):
                        ky, kx = divmod(t, k)
                        y0 = interior - pad + stride * ky
                        x0 = interior - pad + stride * kx
                        st = gfirst and t == 0
                        sp = glast and t == taps - 1
                        for h2 in range(2):
                            nc.tensor.matmul(
                                psum3[:, 16 * h2:16 * h2 + 16, :],
                                fwt[:, t - a, :],
                                zt[:, y0 + 16 * h2:y0 + 16 * h2 + 16,
                                   x0:x0 + 32],
                                start=st, stop=sp)

            def conv_branch(o, m, b, x3, stp, stp_flags):
                """emit one conv branch. stp_flags = (gfirst, glast) for the
                state-psum accumulation group."""
                gfirst, glast = stp_flags
                if o == O_SEP5:  # PE-fused two-stage
                    z1 = z36[0]
                    nc.scalar.activation(z1[:, 2:34, 2:34], x3, ACTF.Relu,
                                         scale=alf[:, m, b, o:o + 1])
                    scr = scrpool.tile([128, 32, 32], F32, tag="scr")
                    fused_stage(fw5a_d, m, z1, 2, 1, 2, 5, scr, True, True)
                    z2 = z36[1]
                    nc.scalar.activation(z2[:, 2:34, 2:34], scr[:], ACTF.Relu)
                    fused_stage(fw5b_d, m, z2, 2, 1, 2, 5, stp, gfirst, glast)
                    return
                if o == O_DIL5:  # PE-fused one-stage
                    z1 = z40[0]
                    nc.scalar.activation(z1[:, 4:36, 4:36], x3, ACTF.Relu,
                                         scale=alf[:, m, b, o:o + 1])
                    fused_stage(fwd5_d, m, z1, 4, 2, 4, 5, stp, gfirst, glast)
                    return
                if o == O_SEP3:
                    bufs, pad, stride, k, t0a, t0b = z34, 1, 1, 3, TAP_OFF["s3a"], TAP_OFF["s3b"]
                    pwa, pwb = PW_SLOT["s3a"], PW_SLOT["s3b"]
                else:  # O_DIL3
                    bufs, pad, stride, k, t0a = z40, 2, 2, 3, TAP_OFF["d3"]
                    pwa = PW_SLOT["d3"]
                interior = (bufs[0].shape[1] - 32) // 2
                i0, i1 = interior, interior + 32
                two_stage = o == O_SEP3
                eng = nc.vector  # scalar_tensor_tensor not supported on GpSimd

                z1 = bufs[0]
                nc.scalar.activation(z1[:, i0:i1, i0:i1], x3, ACTF.Relu,
                                     scale=alf[:, m, b, o:o + 1])
                dwacc = dpool.tile([128, 32, 32], F32R, tag="dwacc")
                dw_chain(eng, z1, dwacc, m, t0a, k, pad, stride, interior)
                if two_stage:
                    scr = scrpool.tile([128, 32, 32], F32, tag="scr")
                    mm_chunks(scr, pw[:, m, pwa, :], dwacc,
                              (True, True, True, True))
                    z2 = bufs[1]
                    nc.scalar.activation(z2[:, i0:i1, i0:i1], scr[:], ACTF.Relu)
                    dwacc2 = dpool.tile([128, 32, 32], F32R, tag="dwacc")
                    dw_chain(eng, z2, dwacc2, m, t0b, k, pad, stride, interior)
                    mm_chunks(stp, pw[:, m, pwb, :], dwacc2,
                              (gfirst, glast, gfirst, glast))
                else:
                    mm_chunks(stp, pw[:, m, pwa, :], dwacc,
                              (gfirst, glast, gfirst, glast))

            def build_pools(j, need_max, need_avg):
                """pool state j -> pooled[:,0,j] (max), pooled[:,1,j] (BN*avg)."""
                x3 = states[:, j]
                if need_max:
                    nc.scalar.copy(xpmax[:, 1:33, 1:33], x3)
                    t = ptmp[0]
                    nc.vector.tensor_max(t[:], xpmax[:, 1:33, 0:32],
                                         xpmax[:, 1:33, 1:33])
                    nc.vector.tensor_max(rmpad[:, 1:33, :], t[:],
                                         xpmax[:, 1:33, 2:34])
                    nc.vector.tensor_max(t[:], rmpad[:, 0:32, :],
                                         rmpad[:, 1:33, :])
                    nc.vector.tensor_max(pooled[:, 0, j], t[:],
                                         rmpad[:, 2:34, :])
                if need_avg:
                    nc.scalar.copy(xpsum[:, 1:33, 1:33], x3)
                    t = ptmp[1]
                    nc.gpsimd.tensor_add(t[:], xpsum[:, 1:33, 0:32],
                                         xpsum[:, 1:33, 1:33])
                    nc.gpsimd.tensor_add(rspad[:, 1:33, :], t[:],
                                         xpsum[:, 1:33, 2:34])
                    nc.gpsimd.tensor_add(t[:], rspad[:, 0:32, :],
                                         rspad[:, 1:33, :])
                    nc.gpsimd.tensor_add(pooled[:, 1, j], t[:],
                                         rspad[:, 2:34, :])
                    nc.gpsimd.tensor_mul(pooled[:, 1, j], pooled[:, 1, j],
                                         rmap[:])

            # which (m,b) use pools, per source state j
            def pool_needs(j, b):
                nm = nav = False
                for step in range(STEPS):
                    if j < 2 + step:
                        m = OFFSETS[step] + j
                        acts = active.get((m, b), ())
                        nm |= O_MAX in acts
                        nav |= O_AVG in acts
                return nm, nav

            # ================= per-sample program =================
            for b in range(BL):
                # ---- preprocess s0, s1 ----
                for inp, xd in ((0, x0_d), (1, x1_d)):
                    scr = scrpool.tile([128, 32, 32], F32, tag="scr")
                    for kc in range(4):
                        xb = xpool.tile([128, HW], F32, tag="xb")
                        nc.sync.dma_start(xb[:], xd[b, kc])
                        xr = xpool.tile([128, HW], F32R, tag="xr")
                        nc.scalar.activation(xr[:], xb[:], ACTF.Relu)
                        for h in range(2):
                            nc.tensor.matmul(
                                scr[:, 16 * h:16 * (h + 1), :],
                                prew[:, inp, kc, :],
                                xr[:, 512 * h:512 * (h + 1)].rearrange(
                                    "p (a c) -> p a c", a=16),
                                start=(kc == 0), stop=(kc == 3))
                    nc.scalar.copy(states[:, inp], scr[:])

                for j in range(2):
                    nm, nav = pool_needs(j, b)
                    build_pools(j, nm, nav)

                # ---- steps ----
                for step in range(STEPS):
                    n_in = 2 + step
                    m0 = OFFSETS[step]
                    stp = pspool.tile([128, 32, 32], F32, tag="stp")
                    # count final matmuls per chunk to set start/stop flags
                    conv_list = []
                    post_list = []
                    for j in range(n_in):
                        m = m0 + j
                        for o in active.get((m, b), ()):
                            if o in (O_SEP3, O_SEP5, O_DIL3, O_DIL5):
                                conv_list.append((o, m, j))
                            else:
                                post_list.append((o, m, j))
                    n_mm = len(conv_list)
                    for i, (o, m, j) in enumerate(conv_list):
                        flags = (i == 0, i == n_mm - 1)
                        conv_branch(o, m, b, states[:, j], stp, flags)
                    if n_mm == 0:
                        nc.vector.memset(stp[:], 0.0)
                    for (o, m, j) in post_list:
                        if o == O_SKIP:
                            src = states[:, j]
                            sc = alf[:, m, b, O_SKIP:O_SKIP + 1]
                        elif o == O_MAX:
                            src = pooled[:, 0, j]
                            sc = alf[:, m, b, O_MAX:O_MAX + 1]
                        else:
                            src = pooled[:, 1, j]
                            sc = alf[:, m, b, O_AVG:O_AVG + 1]
                        nc.vector.scalar_tensor_tensor(
                            stp[:], src, sc, stp[:], op0=ALU.mult, op1=ALU.add)
                    # evacuate state
                    nc.scalar.copy(states[:, 2 + step], stp[:])
                    if step < STEPS - 1:
                        nm, nav = pool_needs(2 + step, b)
                        build_pools(2 + step, nm, nav)

                # ---- output ----
                for i in range(4):
                    nc.sync.dma_start(out_d[b, i],
                                      flat(states[:, 2 + i]))

    nc.compile()
    return nc


def host_prepare(inputs):
    """Returns (in_maps, alphas). in_maps: per-core input dicts."""
    s0, s1 = np.asarray(inputs["s0"]), np.asarray(inputs["s1"])
    gates = np.asarray(inputs["gates"])
    top = int(inputs["top"])
    p = _host_alphas(gates, top)  # [N_MIX, B, 8] fp32, exact zeros

    # prew [128, 2, 4, 128]: prew[ci_local, inp, kc, co] = w[co, kc*128+ci] * BN
    prew = np.empty((128, 2, 4, 128), np.float32)
    for inp, wname in ((0, "pre0_w"), (1, "pre1_w")):
        wmat = np.asarray(inputs[wname]) * BN_SCALE  # [C, C_in]
        for kc in range(4):
            prew[:, inp, kc, :] = wmat[:, 128 * kc:128 * (kc + 1)].T

    # pw [128, N_MIX, N_PW, 128]: pw_out[ci, m, slot, co] = w[m, co, ci] * BN
    pw = np.empty((128, N_MIX, N_PW, 128), np.float32)
    for nm, key in (("s3a", "sep3_pw1"), ("s3b", "sep3_pw2"),
                    ("d3", "dil3_pw")):
        wmat = np.asarray(inputs[key]).astype(np.float32) * BN_SCALE  # [M,Co,Ci]
        pw[:, :, PW_SLOT[nm], :] = wmat.transpose(2, 0, 1)

    # fused tap matrices for PE branches: fw[ci, m, t, co] = pw[m,co,ci]*dw[m,ci,t]
    def fuse(pw_key, dw_key, k):
        pwm = np.asarray(inputs[pw_key]).astype(np.float32) * BN_SCALE  # [M,Co,Ci]
        dwm = np.asarray(inputs[dw_key]).astype(np.float32).reshape(N_MIX, C, k * k)
        pwT = pwm.transpose(2, 0, 1)  # [Ci, M, Co]
        dwT = dwm.transpose(1, 0, 2)  # [Ci, M, taps]
        return (pwT[:, :, None, :] * dwT[:, :, :, None]).astype(np.float32)

    fw5a = fuse("sep5_pw1", "sep5_dw1", 5)
    fw5b = fuse("sep5_pw2", "sep5_dw2", 5)
    fwd5 = fuse("dil5_pw", "dil5_dw", 5)

    # dwt [128, N_MIX, 102]: dwt[c, m, tap]
    dwt = np.empty((128, N_MIX, N_TAPS), np.float32)
    for nm, key, k in (("s3a", "sep3_dw1", 3), ("s3b", "sep3_dw2", 3),
                       ("d3", "dil3_dw", 3)):
        w = np.asarray(inputs[key])  # [N_MIX, C, k, k]
        dwt[:, :, TAP_OFF[nm]:TAP_OFF[nm] + k * k] = (
            w.reshape(N_MIX, C, k * k).transpose(1, 0, 2))

    # rmap: BN * 9 / count (pools computed as straight 3x3 valid-sum)
    cnt = np.zeros((32, 32), np.float32)
    for dy in (-1, 0, 1):
        for dx in (-1, 0, 1):
            ys = slice(max(0, -dy), 32 - max(0, dy))
            cnt[max(0, dy):32 - max(0, -dy),
                max(0, dx):32 - max(0, -dx)] += 1
    rmap_1 = (BN_SCALE / cnt).astype(np.float32)
    rmap = np.broadcast_to(rmap_1, (128, 32, 32)).copy()

    in_maps = []
    for core in range(N_CORES):
        sl = slice(core * BL, (core + 1) * BL)
        alf = p[:, sl, :].copy()  # [N_MIX, BL, 8]
        alf[:, :, O_MAX] *= BN_SCALE
        alf_b = np.broadcast_to(alf, (128,) + alf.shape).copy()
        in_maps.append({
            "x0": s0[sl].reshape(BL, 4, 128, HW).astype(np.float32),
            "x1": s1[sl].reshape(BL, 4, 128, HW).astype(np.float32),
            "prew": prew, "pw": pw, "dwt": dwt,
            "fw5a": fw5a, "fw5b": fw5b, "fwd5": fwd5,
            "alf": alf_b.astype(np.float32), "rmap": rmap,
        })
    return in_maps, p


_prog_cache = {}


def _get_dense_program():
    key = "dense"
    if key not in _prog_cache:
        active = {(m, b): (O_MAX, O_AVG, O_SKIP, O_SEP3, O_SEP5, O_DIL3, O_DIL5)
                  for m in range(N_MIX) for b in range(BL)}
        _prog_cache[key] = build_program(active)
    return _prog_cache[key]


def kernel(**inputs):
    in_maps, _ = host_prepare(inputs)
    nc = _get_dense_program()
    res = run_bass_kernel_spmd(nc, in_maps, core_ids=list(range(N_CORES)))
    out = np.empty((B, 512, H, W), np.float32)
    for core in range(N_CORES):
        o = res.results[core]["out"]  # [BL, 4, 128, HW]
        out[core * BL:(core + 1) * BL] = (
            o.reshape(BL, 512, H, W).astype(np.float32))
    return out

